# revision 1
# baseline (speedup 1.0000x reference)
"""Trainium2 Bass kernel for ContextualAttentionModule.

Data-parallel over batch: 8 samples -> 8 NeuronCores, one sample per core.
Per-core pipeline (C=256, H=W=32, L=1024 patches):
  scores  = <fg_patch(p), (bg_patch(l)+eps)/norm(l)>   via 18 shifted f32r matmuls + eps rank-1 term
  prop    = 3x3 spatial window-sum of scores           (separable DVE adds on padded buffers)
  attn    = softmax over l (no max-subtract; sum via ones-matmul over partitions)
  recov   = conv_transpose(attn, kernels)              via PE-transposed patch bank
  final   = recov*mask/9 + fg*(1-mask)
  out     = concat_g relu(dilated_conv_r(final) + b)   in bf16
"""

import numpy as np

import concourse.bass as bass
import concourse.tile as tile
from concourse import bacc, mybir
from concourse.bass_utils import run_bass_kernel_spmd
from concourse.masks import make_identity

F32 = mybir.dt.float32
F32R = mybir.dt.float32r
BF16 = mybir.dt.bfloat16
U16 = mybir.dt.uint16
AF = mybir.ActivationFunctionType
ALU = mybir.AluOpType

EPS = 1e-7
RATES = (1, 2, 4, 8)
OFFS = [(dy, dx) for dy in range(3) for dx in range(3)]

_CACHE = {}


def build_program(debug=False):
    nc = bacc.Bacc()
    fg_d = nc.declare_dram_parameter("fg", [256, 32, 32], F32, isOutput=False)
    bg_d = nc.declare_dram_parameter("bg", [256, 32, 32], F32, isOutput=False)
    mask_d = nc.declare_dram_parameter("maskrow", [1, 1024], F32, isOutput=False)
    w_d = nc.declare_dram_parameter("wconv", [2, 128, 2304], F32, isOutput=False)
    b_d = nc.declare_dram_parameter("bias", [256, 1], F32, isOutput=False)
    out_d = nc.declare_dram_parameter("out", [256, 32, 32], F32, isOutput=True)
    dbg = {}
    if debug:
        for nm, shp in [("d_bgs", [128, 32, 32]), ("d_scores", [128, 32, 32]),
                        ("d_rn", [128, 8]), ("d_E", [128, 32, 32]),
                        ("d_drow", [1, 1024]), ("d_attn2", [128, 32, 32]),
                        ("d_boxg", [1, 32, 32]), ("d_prec", [128, 512]),
                        ("d_final", [128, 32, 32])]:
            dbg[nm] = nc.declare_dram_parameter(nm, shp, F32, isOutput=True)

    with tile.TileContext(nc) as tc:
        _emit(nc, tc, fg_d, bg_d, mask_d, w_d, b_d, out_d, dbg)
    nc.compile()
    return nc


def _ring_zero(nc, buf, n=34, eng=None):
    """Zero only the 1-wide border ring of a [P, n, n] padded buffer."""
    eng = eng or nc.vector
    eng.memset(buf[:, 0:n:n - 1, :].bitcast(F32), 0.0)
    eng.memset(buf[:, 1:n - 1, 0:n:n - 1].bitcast(F32), 0.0)


def _boxsum(nc, scr, src_pad, dst_flat, eng=None):
    """3x3 SAME window sum: [1,34,34] ring-zero padded -> [1,32,32] flat."""
    eng = eng or nc.vector
    eng.tensor_tensor(scr[:, 1:33, 1:33], src_pad[:, 1:33, 0:32],
                      src_pad[:, 1:33, 1:33], ALU.add)
    eng.tensor_tensor(scr[:, 1:33, 1:33], scr[:, 1:33, 1:33],
                      src_pad[:, 1:33, 2:34], ALU.add)
    eng.tensor_tensor(dst_flat[:], scr[:, 0:32, 1:33],
                      scr[:, 1:33, 1:33], ALU.add)
    eng.tensor_tensor(dst_flat[:], dst_flat[:], scr[:, 2:34, 1:33], ALU.add)


def _emit(nc, tc, fg_d, bg_d, mask_d, w_d, b_d, out_d, dbg=None):
    dbg = dbg or {}
    with (
        tc.tile_pool(name="main", bufs=1) as main,
        tc.tile_pool(name="ps_rec", bufs=1, space="PSUM") as psrec_pool,
    ):
        # ---------------- long-lived tiles ----------------
        fg_pad = [main.tile([128, 34, 34], F32R, name=f"fg_pad{c}") for c in range(2)]
        maskb9 = main.tile([128, 32, 32], F32, name="maskb9")
        invmaskb = main.tile([128, 32, 32], F32, name="invmaskb")
        idR = main.tile([128, 128], F32R, name="idR")
        ones_col = main.tile([128, 1], F32R, name="ones_col")
        epsrow = main.tile([1, 128], F32R, name="epsrow")
        rncol = main.tile([128, 8], F32, name="rncol")
        boxg = main.tile([1, 32, 32], F32R, name="boxg")
        boxs2 = main.tile([1, 32, 32], F32R, name="boxs2")
        wsb = [main.tile([128, 2304], BF16, name=f"wsb{c}") for c in range(2)]
        biasb = [main.tile([128, 1], F32, name=f"biasb{c}") for c in range(2)]

        with tc.tile_pool(name="bgsp", bufs=1) as bgsp:
            bgs = [[None] * 9 for _ in range(2)]

            with tc.tile_pool(name="stage", bufs=1) as stage:
                # ---------- phase 0: load, mask, pad, constants ----------
                onesf0 = stage.tile([128, 1], F32, name="onesf0")
                nc.gpsimd.memset(onesf0[:], 1.0)
                nc.vector.tensor_copy(ones_col[:], onesf0[:])
                msrow = stage.tile([1, 1024], F32, name="msrow", tag="nrow",
                                   bufs=2)
                nc.sync.dma_start(msrow[:], mask_d[:])
                nc.gpsimd.partition_broadcast(
                    invmaskb.rearrange("p a b -> p (a b)"), msrow[:])
                nc.vector.tensor_scalar_mul(maskb9[:], invmaskb[:], 1.0 / 9.0)
                nc.vector.tensor_scalar(
                    out=invmaskb[:], in0=invmaskb[:], scalar1=-1.0, scalar2=1.0,
                    op0=ALU.mult, op1=ALU.add)

                epsf = stage.tile([1, 128], F32, name="epsf")
                nc.gpsimd.memset(epsf[:], EPS)
                nc.vector.tensor_copy(epsrow[:], epsf[:])
                idf = stage.tile([128, 128], F32, name="idf")
                make_identity(nc, idf[:])
                nc.gpsimd.tensor_copy(idR[:], idf[:])

                bg_pad = [stage.tile([128, 34, 34], F32R, name=f"bg_pad{c}")
                          for c in range(2)]
                fflat = [stage.tile([128, 32, 32], F32, name=f"fflat{c}",
                                    tag="eflat", bufs=2) for c in range(2)]
                bflat = [stage.tile([128, 32, 32], F32, name=f"bflat{c}")
                         for c in range(2)]
                for c in range(2):
                    nc.sync.dma_start(fflat[c][:], fg_d[128 * c:128 * (c + 1)])
                    nc.gpsimd.dma_start(bflat[c][:], bg_d[128 * c:128 * (c + 1)])
                for c in range(2):
                    _ring_zero(nc, fg_pad[c])
                    nc.scalar.copy(fg_pad[c][:, 1:33, 1:33], fflat[c][:])
                for c in range(2):
                    _ring_zero(nc, bg_pad[c])
                    nc.vector.tensor_tensor(
                        bg_pad[c][:, 1:33, 1:33], bflat[c][:], invmaskb[:], ALU.mult)

                # shifted masked-bg flats (scores weights + transpose sources)
                k = 0
                for c in range(2):
                    for d, (dy, dx) in enumerate(OFFS):
                        f = bgsp.tile([128, 32, 32], F32R, name=f"bgs{c}_{d}")
                        src = bg_pad[c][:, dy:32 + dy, dx:dx + 32]
                        if k % 3 == 0:
                            nc.scalar.copy(f[:], src)
                        elif k % 3 == 1:
                            nc.vector.tensor_copy(f[:], src)
                        else:
                            nc.gpsimd.tensor_copy(f[:], src)
                        bgs[c][d] = f
                        k += 1

                # ---------- phase 1: patch norms + fg patch sums ----------
                bgsq = [stage.tile([128, 32, 32], F32R, name=f"bgsq{c}",
                                   tag="eflat", bufs=2) for c in range(2)]
                nc.vector.tensor_tensor(bgsq[0][:], bg_pad[0][:, 1:33, 1:33],
                                        bg_pad[0][:, 1:33, 1:33], ALU.mult)
                nc.scalar.square(bgsq[1][:], bg_pad[1][:, 1:33, 1:33])
                ssq = stage.tile([1, 32, 32], F32R, name="ssq")
                s1b = stage.tile([1, 32, 32], F32R, name="s1b")

                with tc.tile_pool(name="ps_rows", bufs=1, space="PSUM") as psr:
                    specs = [
                        (lambda c, r0, r1: fg_pad[c][:, 1 + r0:1 + r1, 1:33],
                         boxg, nc.vector),
                        (lambda c, r0, r1: bg_pad[c][:, 1 + r0:1 + r1, 1:33],
                         s1b, nc.gpsimd),
                        (lambda c, r0, r1: bgsq[c][:, r0:r1, :], ssq, nc.vector),
                    ]
                    for si, (view, dst, beng) in enumerate(specs):
                        rpad = stage.tile([1, 34, 34], F32R, name=f"rpad{si}",
                                          tag="rpad", bufs=2)
                        rscr = stage.tile([1, 34, 34], F32R, name=f"rscr{si}",
                                          tag="rscr", bufs=2)
                        _ring_zero(nc, rpad, eng=beng)
                        _ring_zero(nc, rscr, eng=beng)
                        for ch in range(2):
                            pr = psr.tile([1, 512], F32, name="pr", tag="pr", bufs=2)
                            r0, r1 = 16 * ch, 16 * ch + 16
                            for c in range(2):
                                nc.tensor.matmul(pr[:], ones_col[:], view(c, r0, r1),
                                                 start=(c == 0), stop=(c == 1))
                            nc.scalar.copy(rpad[:, 1 + r0:1 + r1, 1:33], pr[:])
                        _boxsum(nc, rscr, rpad, dst, eng=beng)

                # norm = sqrt(ssq + 2*eps*s1 + 2304*eps^2); rncol[:, t] = 1/norm
                urow = stage.tile([1, 1024], F32, name="urow", tag="nrow", bufs=2)
                nc.vector.scalar_tensor_tensor(
                    out=urow[:], in0=s1b.rearrange("o a b -> o (a b)"),
                    scalar=2.0 * EPS, in1=ssq.rearrange("o a b -> o (a b)"),
                    op0=ALU.mult, op1=ALU.add)
                nc.vector.tensor_scalar_add(urow[:], urow[:], 2304.0 * EPS * EPS)
                sqrow = stage.tile([1, 1024], F32, name="sqrow", tag="nrow", bufs=2)
                nc.scalar.activation(sqrow[:], urow[:], AF.Sqrt)
                rnrow = stage.tile([1, 1024], F32, name="rnrow", tag="nrow", bufs=2)
                nc.vector.reciprocal(rnrow[:], sqrow[:])
                for t in range(8):
                    nc.gpsimd.dma_start(rncol[:, t:t + 1],
                                        rnrow[0:1, 128 * t:128 * (t + 1)])
                if dbg:
                    nc.gpsimd.dma_start(dbg["d_bgs"][:], bgs[0][4][:].bitcast(F32))
                    nc.gpsimd.dma_start(dbg["d_rn"][:], rncol[:])
                    nc.gpsimd.dma_start(dbg["d_boxg"][:], boxg[:].bitcast(F32))
            # ---------- stage pool closed ----------

            # f32r copy of the norm-reciprocal columns (s2 colsum weights)
            rncolR = main.tile([128, 8], F32R, name="rncolR")
            nc.vector.tensor_copy(rncolR[:], rncol[:])

            # padded scores/attn buffers, created after staging frees space
            A = [bgsp.tile([128, 34, 34], F32R, name=f"A{t}") for t in range(8)]
            for t in range(8):
                _ring_zero(nc, A[t], eng=(nc.vector if t % 2 else nc.gpsimd))

            with tc.tile_pool(name="workp", bufs=1) as workp:
                for c in range(2):
                    wstage = workp.tile([128, 2304], F32, name="wstage",
                                        tag="wstage", bufs=1)
                    nc.scalar.dma_start(wstage[:], w_d[c])
                    nc.vector.tensor_copy(wsb[c][:], wstage[:])
                    nc.scalar.dma_start(biasb[c][:], b_d[128 * c:128 * (c + 1)])
                # ---------- phase 3+4: scores, propagation, exp ----------
                W = [workp.tile([128, 34, 34], F32R, name=f"W{i}") for i in range(2)]
                for w in W:
                    _ring_zero(nc, w)

                with (
                    tc.tile_pool(name="ps_sc", bufs=4, space="PSUM") as ps_sc,
                    tc.tile_pool(name="hp", bufs=2) as hp,
                ):
                    for t in range(8):
                        for ch in range(2):
                            psc = ps_sc.tile([128, 512], F32, name="psc", tag="psc")
                            r0 = 16 * ch
                            i = 0
                            for c in range(2):
                                for d, (dy, dx) in enumerate(OFFS):
                                    nc.tensor.matmul(
                                        psc[:],
                                        bgs[c][d].rearrange("p a b -> p (a b)")
                                        [:, 128 * t:128 * (t + 1)],
                                        fg_pad[c][:, r0 + dy:r0 + dy + 16,
                                                  dx:dx + 32],
                                        start=(i == 0), stop=False)
                                    i += 1
                            nc.tensor.matmul(psc[:], epsrow[:],
                                             boxg[:, r0:r0 + 16, :],
                                             start=False, stop=True)
                            # evict raw scores (norm-scale folded into exp)
                            if ch == 0:
                                nc.scalar.copy(A[t][:, 1:17, 1:33], psc[:])
                            else:
                                nc.vector.tensor_copy(
                                    A[t][:, 17:33, 1:33], psc[:])

                        if dbg and t == 0:
                            nc.gpsimd.dma_start(
                                dbg["d_scores"][:], A[0][:, 1:33, 1:33].bitcast(F32))
                        # separable 3x3 window sum -> H, exp -> back into A[t]
                        w = W[t % 2]
                        nc.vector.tensor_tensor(
                            w[:, 1:33, 1:33], A[t][:, 1:33, 0:32],
                            A[t][:, 1:33, 1:33], ALU.add)
                        nc.vector.tensor_tensor(
                            w[:, 1:33, 1:33], w[:, 1:33, 1:33],
                            A[t][:, 1:33, 2:34], ALU.add)
                        H = hp.tile([128, 32, 32], F32R, name="H", tag="H")
                        nc.vector.tensor_tensor(
                            H[:], w[:, 0:32, 1:33], w[:, 1:33, 1:33], ALU.add)
                        nc.vector.tensor_tensor(
                            H[:], H[:], w[:, 2:34, 1:33], ALU.add)
                        nc.scalar.activation(A[t][:, 1:33, 1:33], H[:], AF.Exp,
                                             scale=rncol[:, t:t + 1])
                        if dbg and t == 0:
                            nc.gpsimd.dma_start(
                                dbg["d_E"][:], A[0][:, 1:33, 1:33].bitcast(F32))

                    # ---------- phase 4b: softmax denominator + s2 row ----------
                    if True:  # (accumulators live in the score-psum slots)
                        psd = [ps_sc.tile([1, 512], F32, name=f"psd{ch}",
                                          tag="psc") for ch in range(2)]
                        pss = [ps_sc.tile([1, 512], F32, name=f"pss{ch}",
                                          tag="psc") for ch in range(2)]
                        for t in range(8):
                            for ch in range(2):
                                r0 = 16 * ch
                                nc.tensor.matmul(
                                    psd[ch][:], ones_col[:],
                                    A[t][:, 1 + r0:17 + r0, 1:33],
                                    start=(t == 0), stop=(t == 7))
                                nc.tensor.matmul(
                                    pss[ch][:], rncolR[:, t:t + 1],
                                    A[t][:, 1 + r0:17 + r0, 1:33],
                                    start=(t == 0), stop=(t == 7))
                        rdrow = workp.tile([1, 1024], F32, name="rdrow")
                        s2raw = workp.tile([1, 1024], F32, name="s2raw")
                        for ch in range(2):
                            nc.vector.reciprocal(
                                rdrow[:, 512 * ch:512 * (ch + 1)], psd[ch][:])
                            nc.vector.tensor_copy(
                                s2raw[:, 512 * ch:512 * (ch + 1)], pss[ch][:])
                        # s2 = recipD * sum_l rn*E ; build padded + boxsum now
                        s2_pad = workp.tile([1, 34, 34], F32R, name="s2_pad")
                        _ring_zero(nc, s2_pad, eng=nc.gpsimd)
                        nc.vector.tensor_tensor(
                            s2_pad[:, 1:33, 1:33],
                            s2raw[:].rearrange("o (a b) -> o a b", b=32),
                            rdrow[:].rearrange("o (a b) -> o a b", b=32), ALU.mult)
                        rscr2 = workp.tile([1, 34, 34], F32R, name="rscr2")
                        _ring_zero(nc, rscr2, eng=nc.gpsimd)
                        _boxsum(nc, rscr2, s2_pad, boxs2)
                        if dbg:
                            nc.gpsimd.dma_start(dbg["d_drow"][:], rdrow[:])
                        Db = bgsp.tile([128, 32, 32], F32, name="Db")
                        nc.gpsimd.partition_broadcast(
                            Db.rearrange("p a b -> p (a b)"), rdrow[:])

                    # ---------- phase 5: attn = E * (1/D), in place ----------
                    # (the extra 1/norm kernel-normalization factor is folded
                    #  into the bgT weights at eviction time)
                    for t in range(8):
                        nc.vector.tensor_tensor(
                            A[t][:, 1:33, 1:33], A[t][:, 1:33, 1:33], Db[:],
                            ALU.mult)

            if dbg:
                nc.gpsimd.dma_start(
                    dbg["d_attn2"][:], A[0][:, 1:33, 1:33].bitcast(F32))
            # ---------- phase 6: tconv (contract over l), s2 + bgT interleaved ----------
            prec = [[psrec_pool.tile([128, 512], F32, name=f"prec{c}_{ch}")
                     for ch in range(2)] for c in range(2)]
            with (
                tc.tile_pool(name="ps_tr", bufs=4, space="PSUM") as pstr_pool,
                tc.tile_pool(name="bgTp", bufs=5) as bgTp,
            ):
                blocks = [(c, d) for c in range(2) for d in range(9)]

                def build_bgT(t):
                    bgT = bgTp.tile([128, 2304], F32R, name="bgT", tag="bgT")
                    for grp in range(5):  # 4 transposed blocks per psum tile
                        chunk = blocks[4 * grp:4 * grp + 4]
                        ptr = pstr_pool.tile([128, 512], F32R, name="ptr", tag="ptr")
                        for bi, (c, d) in enumerate(chunk):
                            nc.tensor.transpose(
                                ptr[:, 128 * bi:128 * (bi + 1)],
                                bgs[c][d].rearrange("p a b -> p (a b)")
                                [:, 128 * t:128 * (t + 1)],
                                idR[:])
                        n = len(chunk)
                        nc.scalar.activation(
                            bgT[:, 512 * grp:512 * grp + 128 * n],
                            ptr[:, :128 * n], AF.Copy, scale=rncol[:, t:t + 1])
                    return bgT

                bgTs = {t: build_bgT(t) for t in range(4)}
                for t in range(8):
                    bgT = bgTs.pop(t)
                    for c in range(2):
                        for ch in range(2):
                            for d, (dy, dx) in enumerate(OFFS):
                                z0 = 16 * ch + 2 - dy
                                x0 = 2 - dx
                                nc.tensor.matmul(
                                    prec[c][ch][:],
                                    bgT[:, 128 * (9 * c + d):128 * (9 * c + d + 1)],
                                    A[t][:, z0:z0 + 16, x0:x0 + 32],
                                    start=(t == 0 and d == 0),
                                    stop=(t == 7 and d == 8))
                    if t == 3:
                        # eps term: recovered += eps * ones_c (x) boxs2
                        for c in range(2):
                            for ch in range(2):
                                nc.tensor.matmul(
                                    prec[c][ch][:], epsrow[:],
                                    boxs2[:, 16 * ch:16 * ch + 16, :],
                                    start=False, stop=False)
                    if t + 4 < 8:
                        bgTs[t + 4] = build_bgT(t + 4)
        # ---------- bgsp closed (bgs + A freed) ----------

        if dbg:
            with tc.tile_pool(name="dbgp", bufs=1) as dbgp:
                dtmp = dbgp.tile([128, 512], F32, name="dtmp")
                nc.vector.tensor_copy(dtmp[:], prec[0][0][:])
                nc.gpsimd.dma_start(dbg["d_prec"][:], dtmp[:])
        with tc.tile_pool(name="late", bufs=1) as late:
            # ---------- phase 7: final = recov*mask/9 + fg*(1-mask) ----------
            final_pad = [late.tile([128, 48, 48], BF16, name=f"final_pad{c}")
                         for c in range(2)]
            for c in range(2):
                nc.vector.memset(final_pad[c][:, 0:8, :].bitcast(U16), 0)
                nc.vector.memset(final_pad[c][:, 40:48, :].bitcast(U16), 0)
                nc.gpsimd.memset(final_pad[c][:, 8:40, 0:8].bitcast(U16), 0)
                nc.gpsimd.memset(final_pad[c][:, 8:40, 40:48].bitcast(U16), 0)
            fscr = [late.tile([128, 32, 32], F32, name=f"fscr{i}") for i in range(4)]
            for c in range(2):
                for ch in range(2):
                    r0 = 16 * ch
                    nc.vector.tensor_tensor(
                        fscr[c][:, r0:r0 + 16, :], prec[c][ch][:],
                        maskb9[:, r0:r0 + 16, :], ALU.mult)
                nc.gpsimd.tensor_tensor(fscr[2 + c][:], fg_pad[c][:, 1:33, 1:33],
                                        invmaskb[:], ALU.mult)
                nc.vector.tensor_tensor(final_pad[c][:, 8:40, 8:40],
                                        fscr[c][:], fscr[2 + c][:], ALU.add)

            # ---------- phase 8: dilated convs (bf16) ----------
            out_sb = [late.tile([128, 32, 32], F32, name=f"out_sb{c}")
                      for c in range(2)]

            with tc.tile_pool(name="ps_o", bufs=3, space="PSUM") as pso_pool:
                for ct_out in range(2):
                    for ch in range(2):
                        pso = pso_pool.tile([128, 512], F32, name="pso", tag="pso")
                        for half in range(2):
                            g = 2 * ct_out + half
                            r = RATES[g]
                            i = 0
                            for c in range(2):
                                for d, (dy, dx) in enumerate(OFFS):
                                    oy = 8 + r * (dy - 1) + 16 * ch
                                    ox = 8 + r * (dx - 1)
                                    woff = 576 * g + 64 * (3 * dy + dx)
                                    nc.tensor.matmul(
                                        pso[64 * half:64 * half + 64, :],
                                        wsb[c][:, woff:woff + 64],
                                        final_pad[c][:, oy:oy + 16, ox:ox + 32],
                                        start=(i == 0), stop=(i == 17),
                                        tile_position=(0, 64 * half))
                                    i += 1
                        nc.scalar.activation(
                            out_sb[ct_out][:, 16 * ch:16 * ch + 16, :],
                            pso[:].rearrange("p (a b) -> p a b", b=32),
                            AF.Relu, bias=biasb[ct_out][:])
                        nc.sync.dma_start(
                            out_d[128 * ct_out:128 * (ct_out + 1),
                                  16 * ch:16 * ch + 16, :],
                            out_sb[ct_out][:, 16 * ch:16 * ch + 16, :])
            if dbg:
                ftmp = late.tile([128, 32, 32], F32, name="ftmp")
                nc.scalar.copy(ftmp[:], final_pad[0][:, 8:40, 8:40])
                nc.gpsimd.dma_start(dbg["d_final"][:], ftmp[:])



def _get_nc():
    if "nc" not in _CACHE:
        _CACHE["nc"] = build_program()
    return _CACHE["nc"]


def kernel(foreground, mask, background, conv_w, conv_b):
    nc = _get_nc()
    fg = np.ascontiguousarray(foreground, dtype=np.float32)
    bg = np.ascontiguousarray(background, dtype=np.float32)
    maskrow = np.ascontiguousarray(mask.reshape(1, 1024), dtype=np.float32)
    # conv_w [4,64,256,3,3] -> [c, g, dy, dx, o] -> [2, 128, 2304]
    wre = np.ascontiguousarray(
        conv_w.astype(np.float32).transpose(2, 0, 3, 4, 1).reshape(2, 128, 2304))
    bias = np.ascontiguousarray(conv_b.astype(np.float32).reshape(256, 1))
    in_maps = [
        {"fg": fg[i], "bg": bg[i], "maskrow": maskrow, "wconv": wre, "bias": bias}
        for i in range(8)
    ]
    res = run_bass_kernel_spmd(nc, in_maps, list(range(8)))
    return np.stack([res.results[i]["out"] for i in range(8)], axis=0)


if __name__ == "__main__":
    build_program()
    print("build ok")

